# revision 1
# baseline (speedup 1.0000x reference)
"""Location-sensitive attention TRN2 Bass kernel (v2).

Data-parallel over batch: B=64 sharded as 8 per NeuronCore across 8 cores;
parameters replicated. Per core:

  query   = decoder_hidden @ Wq                     [8, 128]   (prep, on PE)
  keys    = encoder_outputs @ Wk                    [8, 2048, 128]
  loc     = conv1d(prev_attention) ; loc_term = loc @ Wl
  energy  = tanh(keys + query + loc_term) @ v       [8, 2048]
  out     = softmax(energy, axis=T)

The kernel is HBM-read bound (32MB f32 of encoder_outputs per core);
everything else is structured to hide under that stream (~87-97us/rep
measured, vs ~88us roofline at ~2.9TB/s chip HBM over 8 cores).

Design notes:
 * enc arrives [tok, feat]; PE contracts over partitions, so enc is
   transposed on-chip with regular fp16 identity matmuls (transpose-mode
   throttles the PE clock — measured 4.8x slower end-to-end).
 * enc loads: two h-major slabs, quarter-slab DMAs [128p, 2b, 8 tok,
   512f] with f32->fp16 cast (16KB contiguous HBM reads per partition —
   the pattern previously measured at 422GB/s), split over two SWDGE
   queues. Partition p holds tokens 16p + 8h + s; the final
   normalization multiply un-permutes via its read AP.
 * Cross-segment software pipeline over 32 (h, j, b) segments:
   T-burst(s) | wk-burst(s-1) | v-dot(s-3) on PE; PSUM->SBUF copies on
   DVE/ACT (GPSIMD cannot touch PSUM); tanh(s-2) + exp on ACT. j-passes
   interleave at batch-pair granularity so each quarter-slab is consumed
   in 4 consecutive segments and the DMA stream stays busy to the end.
 * Transposed tiles use two-bank [128, 1024] PSUM tiles: one copy
   instruction moves 2 c-chunks (amortizes the fixed access latency).
 * exp rows: the v-dot matmuls for all 8 batches accumulate into one
   [8, 512] PSUM tile per 512-token segment, using per-batch column-
   selector weights (v8[:, b, :] = v in column b, zeros elsewhere). One
   ACT exp per segment (with accum_out emitting the partial softmax
   denominator for free) replaces the 32 tiny per-row HWDGE DMAs of v1,
   which nearly saturated the SP sequencer.
 * All weight loads ride the two HWDGE queues as f32 (cast on-chip); the
   SWDGE queues carry only the pa gather + enc stream.
 * conv+Wl fold into WW [31, 128] (ww[k,:] = sum_c conv_w[c,k]*Wl[c,:]),
   applied against a [31, 512] shifted-window view of prev_attention
   (f32 DRAM round-trip over HWDGE, casting SWDGE gather). conv_b @ Wl
   joins query in the per-partition tanh bias. exp needs no
   max-subtraction: |energy| <= ||v||_1 ~ 11, inside fp32 exp range.
 * Final normalize splits 3 ways (DVE/ACT/Pool) and the output rides
   both HWDGE queues.
"""
import sys

sys.path.insert(0, "/opt/trn_rl_repo")

from contextlib import ExitStack

import numpy as np

import concourse.bass as bass
import concourse.tile as tile
from concourse import bacc, mybir
from concourse.bass_utils import run_bass_kernel_spmd
from concourse.masks import make_identity

B, T, ENC_DIM = 64, 2048, 512
Q_DIM, ATTN, CH, KS, PAD = 256, 128, 32, 31, 15
N_CORES = 8
BL = B // N_CORES  # 8 batches per core

f32 = mybir.dt.float32
fp16 = mybir.dt.float16
AF = mybir.ActivationFunctionType


N_SWDGE_Q = 2  # split the enc stream over multiple SWDGE queues


def build(reps: int = 1, nq: int = N_SWDGE_Q):
    nc = bacc.Bacc("TRN2", target_bir_lowering=False, debug=False,
                   num_devices=N_CORES, num_swdge_queues=nq)

    enc_d = nc.dram_tensor("encoder_outputs", [BL, T, ENC_DIM], f32,
                           kind="ExternalInput").ap()
    dh_d = nc.dram_tensor("decoder_hidden", [BL, Q_DIM], f32,
                          kind="ExternalInput").ap()
    pa_d = nc.dram_tensor("prev_attention", [BL, T], f32,
                          kind="ExternalInput").ap()
    wq_d = nc.dram_tensor("Wq", [Q_DIM, ATTN], f32, kind="ExternalInput").ap()
    wk_d = nc.dram_tensor("Wk", [ENC_DIM, ATTN], f32, kind="ExternalInput").ap()
    cw_d = nc.dram_tensor("conv_w", [CH, 1, KS], f32, kind="ExternalInput").ap()
    cb_d = nc.dram_tensor("conv_b", [CH], f32, kind="ExternalInput").ap()
    wl_d = nc.dram_tensor("Wl", [CH, ATTN], f32, kind="ExternalInput").ap()
    v_d = nc.dram_tensor("v", [ATTN], f32, kind="ExternalInput").ap()
    out_d = nc.dram_tensor("out", [BL, T], f32, kind="ExternalOutput").ap()

    # internal DRAM scratch for the zero-padded prev_attention rows
    pa_pad_d = nc.dram_tensor("pa_pad", [BL, T + 32], f32).ap()

    with tile.TileContext(nc) as tc, ExitStack() as ctx:
        singles = ctx.enter_context(tc.tile_pool(name="singles", bufs=1))
        sb_enc = ctx.enter_context(tc.tile_pool(name="enc", bufs=3))
        sb_xt = ctx.enter_context(tc.tile_pool(name="xt", bufs=4))
        sb_tanh = ctx.enter_context(tc.tile_pool(name="tanh", bufs=3))
        sb_sm = ctx.enter_context(tc.tile_pool(name="sm", bufs=2))
        ps_xt = ctx.enter_context(tc.tile_pool(name="ps_xt", bufs=2, space="PSUM"))
        ps_o = ctx.enter_context(tc.tile_pool(name="ps_o", bufs=2, space="PSUM"))
        ps_e = ctx.enter_context(tc.tile_pool(name="ps_e", bufs=2, space="PSUM"))

        # ---------- prev_attention staging ----------
        # pad edges + center stay f32 and ride HWDGE (DRAM->DRAM), keeping
        # the SWDGE queue head free for the first enc half-slab; the
        # shifted-window gather casts f32->fp16 on SWDGE right after it.
        zeros_sb = singles.tile([BL, 32], f32)
        nc.vector.memset(zeros_sb, 0.0)
        nc.sync.dma_start(pa_pad_d[:, PAD:PAD + T], pa_d)
        nc.sync.dma_start(pa_pad_d[:, 0:PAD], zeros_sb[:, 0:PAD])
        nc.sync.dma_start(pa_pad_d[:, PAD + T:T + 32], zeros_sb[:, 0:32 - PAD])
        pa_sh = singles.tile([KS, BL, T], fp16)

        def pa_gather():
            # pa_sh[k, b, t] = pa_pad[b, t + k]  (k=0..30), casts to fp16
            nc.gpsimd.dma_start(
                pa_sh,
                bass.AP(tensor=pa_pad_d.tensor, offset=0,
                        ap=[[1, KS], [T + 32, BL], [1, T]]))

        # ---------- weights: f32 over both HWDGE queues, cast on-chip ----
        # (split across sync+scalar so every small transfer beats the first
        # enc half-slab to the shared DMA engines)
        wk_f = singles.tile([128, 4, ATTN], f32)
        nc.sync.dma_start(wk_f, wk_d.rearrange("(c k) a -> k c a", c=4))
        wq_f = singles.tile([128, 2, ATTN], f32)
        nc.scalar.dma_start(wq_f, wq_d.rearrange("(c k) a -> k c a", c=2))
        dh_f = singles.tile([BL, Q_DIM], f32)
        nc.sync.dma_start(dh_f, dh_d)
        cw_f = singles.tile([CH, KS], f32)
        nc.scalar.dma_start(cw_f, cw_d.rearrange("c o k -> c (o k)"))
        cb_f = singles.tile([CH, 1], f32)
        nc.scalar.dma_start(
            cb_f, bass.AP(tensor=cb_d.tensor, offset=0, ap=[[1, CH], [1, 1]]))
        wl_f = singles.tile([CH, ATTN], f32)
        nc.sync.dma_start(wl_f, wl_d)
        v_f = singles.tile([ATTN, 1], f32)
        nc.scalar.dma_start(
            v_f, bass.AP(tensor=v_d.tensor, offset=0, ap=[[1, ATTN], [1, 1]]))

        # ---------- constants ----------
        ident_f = singles.tile([128, 128], f32)
        make_identity(nc, ident_f)
        ident = singles.tile([128, 128], fp16)
        nc.scalar.copy(ident, ident_f)

        wk_sb = singles.tile([128, 4, ATTN], fp16)
        nc.vector.tensor_copy(wk_sb, wk_f)

        # ---------- prep on PE (f32, sizes are tiny) ----------
        # dhT [256, 8] via two transpose-matmuls of dh [8, 256]
        dhT_ps = ps_xt.tile([128, 2, BL], f32, tag="xt", name="dhT_ps")
        for c in range(2):
            nc.tensor.matmul(dhT_ps[:, c, :], dh_f[:, c * 128:(c + 1) * 128],
                             ident_f[0:BL, 0:BL], start=True, stop=True)
        dhT_sb = singles.tile([128, 2, BL], f32)
        nc.scalar.copy(dhT_sb, dhT_ps)

        # queryT [A, 8] = Wq.T @ dhT  (accumulate 2 chunks of q-dim)
        qt_ps = ps_o.tile([ATTN, BL], f32, tag="o", name="qt_ps")
        for c in range(2):
            nc.tensor.matmul(qt_ps, wq_f[:, c, :], dhT_sb[:, c, :],
                             start=(c == 0), stop=(c == 1))
        # cbwl [A, 1] = Wl.T @ conv_b ; joins query in the tanh bias
        cbwl_ps = ps_xt.tile([ATTN, 1], f32, tag="xt", name="cbwl_ps")
        nc.tensor.matmul(cbwl_ps, wl_f, cb_f, start=True, stop=True)
        qt_sb = singles.tile([ATTN, BL], f32)
        nc.vector.tensor_scalar_add(qt_sb, qt_ps, cbwl_ps)

        # WW [31, A]: ww[k, :] = sum_c conv_w[c, k] * Wl[c, :]
        ww_ps = ps_o.tile([KS, ATTN], f32, tag="o", name="ww_ps")
        nc.tensor.matmul(ww_ps, cw_f, wl_f, start=True, stop=True)
        ww_sb = singles.tile([KS, ATTN], fp16)
        nc.vector.tensor_copy(ww_sb, ww_ps)

        # v8[:, b, :] = v in column b, zeros elsewhere (batch-selector for
        # the energy dot: out[8,512] accumulates row b = v . tanh_b)
        v8_f = singles.tile([ATTN, BL, BL], f32)
        nc.gpsimd.memset(v8_f, 0.0)
        _d = v8_f[:, :, :]
        nc.gpsimd.tensor_copy(
            bass.AP(tensor=_d.tensor, offset=_d.offset,
                    ap=[_d.ap[0], [BL + 1, BL]]),
            bass.AP(tensor=v_f.tensor, offset=v_f.offset,
                    ap=[v_f.ap[0], [0, BL]]))
        v8_sb = singles.tile([ATTN, BL, BL], fp16)
        nc.gpsimd.tensor_copy(v8_sb, v8_f)


        # ---------- main loop ----------
        # Two h-major slabs: slab h holds tokens t = 16p + 8h + s for all 8
        # batches (8 consecutive tokens -> 16KB contiguous HBM reads per
        # partition). Each slab is consumed in two j-passes; segment
        # (h, j, b) computes the [A, 512] energy tile whose col 128q + p
        # <-> token t = 16p + 8h + 4j + q.
        # Cross-segment software pipeline:
        #   T-burst(s) | wk-burst(s-1) | v8(s-3) on PE;
        #   copies(s) on DVE/ACT; tanh(s-2), exp(+accum) on ACT.
        def slab_dma(h, b0, rep, slabs, nb=2):
            # one quarter-slab (2 batches, or a 1-batch eighth in the
            # prologue) per tile so consumers only wait on the DMA that
            # covers their batch
            enc_sb = sb_enc.tile([128, nb, 8, ENC_DIM], fp16, tag="enc",
                                 bufs=7, name=f"enc_{rep}_{h}_{b0}",
                                 padded_shape=[128, 2, 8, ENC_DIM])
            inst = nc.gpsimd.dma_start(
                enc_sb,
                bass.AP(tensor=enc_d.tensor,
                        offset=b0 * T * ENC_DIM + 4096 * h,
                        ap=[[16 * ENC_DIM, 128], [T * ENC_DIM, nb],
                            [ENC_DIM, 8], [1, ENC_DIM]]))
            qn = (b0 // 2) % nq
            if qn:
                inst.ins.queue = f"qPoolDynamic{qn}"
            for i in range(nb):
                slabs[(h, b0 + i)] = (enc_sb, i)

        for rep in range(reps):
            exp_sb = sb_sm.tile([BL, T], f32, tag="exp", bufs=1)
            sums4 = sb_sm.tile([BL, 4], f32, tag="sums4")
            slabs = {}
            if rep == 0:
                # prologue: per-batch eighths up front so compute starts on
                # the first 1MB; the pa gather slots in right after
                slab_dma(0, 0, rep, slabs, nb=1)
                pa_gather()
                slab_dma(0, 1, rep, slabs, nb=1)
                for b0 in (2, 4, 6):
                    slab_dma(0, b0, rep, slabs)
            else:
                for b0 in (0, 2, 4, 6):
                    slab_dma(0, b0, rep, slabs)

            s1 = s2 = s3 = None  # pipeline stages: (hj, b, payload)
            e_tiles = {}

            def t_burst(hj, b, enc_entry, seg_idx):
                # two-bank PSUM tiles: one copy instruction moves 2 c-chunks
                j = hj % 2
                enc_sb, bi = enc_entry
                xt_tiles = []
                for cp in range(2):
                    xt_ps = ps_xt.tile([128, 1024], f32, tag="xt")
                    for cc in range(2):
                        c = 2 * cp + cc
                        for q in range(4):
                            nc.tensor.matmul(
                                xt_ps[:, cc * 512 + q * 128:
                                      cc * 512 + (q + 1) * 128],
                                enc_sb[:, bi, 4 * j + q,
                                       c * 128:(c + 1) * 128],
                                ident, start=True, stop=True)
                    xt_sb = sb_xt.tile([128, 1024], fp16, tag="xts")
                    if cp == 0 or seg_idx % 4 == 3:
                        nc.vector.tensor_copy(xt_sb, xt_ps)
                    else:
                        nc.scalar.copy(xt_sb, xt_ps)
                    xt_tiles.append(xt_sb)
                return xt_tiles

            def wk_burst(hj, b, xt_tiles):
                out_ps = ps_o.tile([ATTN, 512], f32, tag="o")
                for c in range(4):
                    nc.tensor.matmul(out_ps, wk_sb[:, c, :],
                                     xt_tiles[c // 2]
                                     [:, (c % 2) * 512:(c % 2 + 1) * 512],
                                     start=(c == 0), stop=False)
                _sl = pa_sh[:, b, 4 * hj:]
                pa_slice = bass.AP(tensor=_sl.tensor, offset=_sl.offset,
                                   ap=[_sl.ap[0], [1, 4], [16, 128]])
                nc.tensor.matmul(out_ps, ww_sb, pa_slice,
                                 start=False, stop=True)
                return out_ps

            def v8_step(hj, b, tanh_sb):
                if b == 0:
                    e_tiles[hj] = ps_e.tile([BL, 512], f32, tag="e",
                                            name=f"e_{rep}_{hj}")
                nc.tensor.matmul(e_tiles[hj], v8_sb[:, b, :], tanh_sb,
                                 start=(b == 0), stop=(b == BL - 1))

            def seg(item):
                nonlocal s1, s2, s3
                if s1 is not None:
                    hh, bb, xts = s1
                    s1 = (hh, bb, wk_burst(hh, bb, xts))
                if s3 is not None:
                    hh, bb, tsb = s3
                    v8_step(hh, bb, tsb)
                if s2 is not None:
                    hh, bb, ops = s2
                    tanh_sb = sb_tanh.tile([ATTN, 512], fp16, tag="tanh")
                    nc.scalar.activation(tanh_sb, ops, AF.Tanh,
                                         bias=qt_sb[:, bb:bb + 1])
                    s2 = (hh, bb, tanh_sb)
                if s3 is not None and s3[1] == BL - 1:
                    hh = s3[0]
                    # exp emits its own partial row-sum via the accumulator
                    nc.scalar.activation(
                        exp_sb[:, hh * 512:(hh + 1) * 512], e_tiles[hh],
                        AF.Exp, accum_out=sums4[:, hh:hh + 1])
                s3, s2, s1 = s2, s1, item

            # pair-interleaved j-passes: each quarter-slab is fully consumed
            # in 4 consecutive segments, so the DMA stream stays busy until
            # near the end of the rep (minimal post-DMA tail)
            si = 0
            for h in range(2):
                for bp in (0, 2, 4, 6):
                    if h == 0:
                        # prefetch slab 1 while slab 0's pass runs
                        slab_dma(1, bp, rep, slabs)
                    for j in range(2):
                        for b in (bp, bp + 1):
                            seg((2 * h + j, b,
                                 t_burst(2 * h + j, b, slabs[(h, b)], si)))
                            si += 1
            for _ in range(3):  # drain
                seg(None)

            # softmax normalization over T, batched across the 8 rows;
            # per-h partial sums landed during the pipeline
            sums = sb_sm.tile([BL, 1], f32, tag="sums")
            nc.vector.reduce_sum(sums, sums4, axis=mybir.AxisListType.X)
            inv = sb_sm.tile([BL, 1], f32, tag="inv")
            nc.vector.reciprocal(inv, sums)
            o_sb = sb_sm.tile([BL, T], f32, tag="osb", bufs=1)
            # un-permute: natural token t = 16p + 4h + q reads storage
            # index h*512 + q*128 + p; 3-way split + both HWDGE queues
            # (A/B-measured faster than a Pool-free 2-way split: the
            # shorter serial tail beats keeping the Pool SEQ clear)
            _e = exp_sb[:, :]
            splits = [(0, 48, nc.vector.tensor_scalar_mul, nc.sync),
                      (48, 48, lambda o, i_, s: nc.scalar.activation(
                          o, i_, AF.Copy, scale=s), nc.scalar),
                      (96, 32, nc.gpsimd.tensor_scalar_mul, nc.sync)]
            for p0, np_, mul, q in splits:
                perm = bass.AP(
                    tensor=_e.tensor, offset=_e.offset + p0,
                    ap=[_e.ap[0], [1, np_], [512, 4], [128, 4]])
                mul(o_sb[:, 16 * p0:16 * (p0 + np_)], perm, inv)
                q.dma_start(out_d[:, 16 * p0:16 * (p0 + np_)],
                            o_sb[:, 16 * p0:16 * (p0 + np_)])

    nc.compile()
    return nc


_cache = {}


def _get(reps: int = 1):
    if reps not in _cache:
        _cache[reps] = build(reps)
    return _cache[reps]


def _in_maps(inputs):
    enc = np.ascontiguousarray(np.asarray(inputs["encoder_outputs"], dtype=np.float32))
    dh = np.ascontiguousarray(np.asarray(inputs["decoder_hidden"], dtype=np.float32))
    pa = np.ascontiguousarray(np.asarray(inputs["prev_attention"], dtype=np.float32))
    rep = {k: np.ascontiguousarray(np.asarray(inputs[k], dtype=np.float32))
           for k in ("Wq", "Wk", "conv_w", "conv_b", "Wl", "v")}
    maps = []
    for i in range(N_CORES):
        s = slice(i * BL, (i + 1) * BL)
        maps.append({"encoder_outputs": enc[s], "decoder_hidden": dh[s],
                     "prev_attention": pa[s], **rep})
    return maps


def kernel(**inputs) -> np.ndarray:
    nc = _get(1)
    res = run_bass_kernel_spmd(nc, _in_maps(inputs), list(range(N_CORES)))
    return np.concatenate([res.results[i]["out"] for i in range(N_CORES)],
                          axis=0).astype(np.float32)


if __name__ == "__main__":
    rng = np.random.default_rng(0)
    ins = {
        "encoder_outputs": rng.standard_normal((B, T, ENC_DIM), dtype=np.float32),
        "decoder_hidden": rng.standard_normal((B, Q_DIM), dtype=np.float32),
        "prev_attention": rng.random((B, T), dtype=np.float32),
        "Wq": (rng.standard_normal((Q_DIM, ATTN), dtype=np.float32) / np.sqrt(Q_DIM)),
        "Wk": (rng.standard_normal((ENC_DIM, ATTN), dtype=np.float32) / np.sqrt(ENC_DIM)),
        "conv_w": (rng.standard_normal((CH, 1, KS), dtype=np.float32) / np.sqrt(KS)),
        "conv_b": np.zeros(CH, dtype=np.float32),
        "Wl": (rng.standard_normal((CH, ATTN), dtype=np.float32) / np.sqrt(CH)),
        "v": (rng.standard_normal(ATTN, dtype=np.float32) / np.sqrt(ATTN)),
    }
    out = kernel(**ins)
    print("kernel output", out.shape, out.dtype, "row sums ~1:",
          np.allclose(out.sum(axis=1), 1.0, atol=1e-3))

